# revision 32
# baseline (speedup 1.0000x reference)
"""Trainium2 Bass kernel for nn_DotProductAttention_6030134084023.

reference: softmax(mask(Q @ K^T / sqrt(64), valid_lens)) @ V
  query/key/value: [64, 1024, 64] f32, valid_lens: [64] int32 -> [64, 1024, 64] f32

Strategy
--------
Batch dim sharded across the 8 NeuronCores. The host sorts batches by
valid_len (descending) and deals them round-robin, so slot s on every core
holds similar-length batches; the kernel is compiled per call with a
per-slot chunk count (chunks past a slot's max length have an all-zero
mask so skipping them is exactly lossless; the kernel recompiles for
whatever valid_lens it receives — correctness never depends on the
specialization).

All matmuls run in bf16 (1 cycle/row on the PE vs 3 for fp32 HIGH mode);
inputs are cast and laid out on the host:

  qt/kt: [BPC, 128, S] bf16, Q^T/K^T duplicated into both 64-row halves
         so row-packed K=64 matmul pairs (tile_position (0,0)/(64,0))
         run concurrently from one SBUF tile, one DMA per tensor.
  vm:    [BPC, S, 65] bf16 = [V * mask, mask] — the valid_lens mask is
         applied to V on the host, so scores need no masking on device
         and UT row 64 accumulates the masked softmax denominator.

Per-core dataflow per batch in the "S^T orientation" (k on SBUF
partitions, q on the free dim; no on-device transposes):

  ST[k, q]  = KT_chunk.T @ QT      PE bf16; two k-chunks concurrently
  EST       = exp(0.125 * ST)      one ScalarE op per [128, 1024] group
  UT[d', q] += Vm_chunk.T @ EST    PE bf16 K=128, PSUM-accumulated

Postprocess per (batch, q-half): recip = 1/UT[64, :] (DVE, PSUM in),
broadcast recip to 64 partitions via a stride-0-partition SBUF DMA,
OT = UT[0:64, :] * bc (DVE), one output DMA per batch.
"""

import numpy as np
import ml_dtypes

import concourse.bass as bass
import concourse.bacc as bacc
import concourse.tile as tile
from concourse import mybir
from concourse import bass_utils

F32 = mybir.dt.float32
BF16 = mybir.dt.bfloat16
I16 = mybir.dt.int16
AF = mybir.ActivationFunctionType
ALU = mybir.AluOpType

# Schraudolph fast-exp constants emitting bf16 bit patterns via the DVE's
# round-to-nearest f32->i16 convert: exp(x/8) ~= bf16_bits(round(x*A + B)).
# Max ~3% rel error; applied only to long-slot score groups where the
# softmax is diffuse and the error washes out (measured 5.6e-3 end to end).
SCHRA_A = float(128.0 * np.log2(np.e) / 8.0)
SCHRA_B = float(16256.0 - 4.5)

NCORES = 8
B = 64
S = 1024
D = 64
BPC = B // NCORES  # 8 batch slots per core
KC = S // 128  # 8 k-chunks of 128
QH = 512  # q-half (max matmul moving dim per PSUM bank)

NPBF16 = ml_dtypes.bfloat16

_BUILD_CACHE = {}


def _build(nreals):
    """nreals[s]: number of 128-key chunks with any valid key for slot s."""
    nc = bacc.Bacc("TRN2", target_bir_lowering=False, debug=False, num_devices=NCORES)
    qt = nc.dram_tensor("qt", [BPC, 128, S], BF16, kind="ExternalInput").ap()
    kt = nc.dram_tensor("kt", [BPC, 128, S], BF16, kind="ExternalInput").ap()
    vm = nc.dram_tensor("vm", [BPC, S, D + 1], BF16, kind="ExternalInput").ap()
    # unnormalized output: rows 0:64 = V'^T @ EST, row 64 = softmax denom;
    # the host divides (free) during the gather
    ot = nc.dram_tensor("ot", [BPC, D + 1, S], F32, kind="ExternalOutput").ap()

    with tile.TileContext(nc) as tc:
        with (
            tc.tile_pool(name="qk", bufs=3) as qkp,
            tc.tile_pool(name="vmp", bufs=3) as vmp,
            tc.tile_pool(name="estp", bufs=8) as estp,
            tc.tile_pool(name="post", bufs=3) as postp,
            tc.tile_pool(name="stp", bufs=3, space="PSUM") as stp,
            tc.tile_pool(name="utp", bufs=1, space="PSUM") as utp,
        ):
            # alternate long/short slots so short-batch postprocess tails
            # hide under the next long batch's compute; end with shortest
            slot_order = [5, 0, 4, 1, 6, 2, 3, 7]
            # STs run LOOKAHEAD pairs ahead of the est-waiting UTs in PE's
            # in-order queue, so both exp engines (ACT + DVE) stay fed
            LOOKAHEAD = 1
            pendq = []
            for bi, b in enumerate(slot_order):
                nreal = nreals[b]
                kw = nreal * 128
                qt2 = qkp.tile([128, S], BF16, tag="qt")
                kt2 = qkp.tile([128, S], BF16, tag="kt")
                # k first, then q; split the first slot's q so its first ST
                # pair starts as early as possible
                # q on the gpsimd queue so its transfer runs in parallel
                # with k/v on the sync queue
                nc.sync.dma_start(out=kt2[:, 0:kw], in_=kt[b, :, 0:kw])
                if bi == 0:
                    nc.gpsimd.dma_start(out=qt2[:, 0:QH], in_=qt[b, :, 0:QH])
                    nc.gpsimd.dma_start(out=qt2[:, QH:S], in_=qt[b, :, QH:S])
                else:
                    nc.gpsimd.dma_start(out=qt2[:], in_=qt[b])

                # all V' chunks of the batch in one strided DMA:
                # vm_all[p, kc*65 + j] = vm[b, kc*128 + p, j]
                vm_all = vmp.tile([128, nreal * (D + 1)], BF16, tag="vm")
                vsrc = vm[b]  # [S, D+1]
                nc.sync.dma_start(
                    out=vm_all[:],
                    in_=bass.AP(
                        tensor=vsrc.tensor,
                        offset=vsrc.offset,
                        ap=[[D + 1, 128], [128 * (D + 1), nreal], [1, D + 1]],
                    ),
                )

                npairs = (nreal + 1) // 2
                uts = [utp.tile([D + 1, QH], F32, tag=f"ut{h}", name=f"ut{h}")
                       for h in range(2)]

                def emit_ut(p, ests, uts=uts, nreal=nreal, vm_all=vm_all):
                    # both halves back-to-back per chunk: shared vm weights
                    for kcl in range(2):
                        kc = 2 * p + kcl
                        if kc >= nreal:
                            continue
                        for h in range(2):
                            nc.tensor.matmul(
                                uts[h][:],
                                vm_all[:, kc * (D + 1) : (kc + 1) * (D + 1)],
                                ests[h][:, kcl * QH : (kcl + 1) * QH],
                                start=(p == 0 and kcl == 0),
                                stop=(kc == nreal - 1),
                            )

                def postprocess(uts=uts, b=b):
                    # one PSUM->SBUF eviction per engine so neither exp
                    # queue eats both copies at the batch boundary
                    for h in range(2):
                        hs = slice(h * QH, (h + 1) * QH)
                        osb = postp.tile([D + 1, QH], F32, tag=f"osb{h}")
                        if h == 0:
                            nc.scalar.copy(out=osb[:], in_=uts[h][:])
                        else:
                            nc.vector.tensor_copy(out=osb[:], in_=uts[h][:])
                        nc.sync.dma_start(out=ot[b, :, hs], in_=osb[:])

                for p in range(npairs):
                    c0 = 2 * p * 128
                    c1 = (2 * p + 1) * 128
                    sts = [stp.tile([128, 2 * QH], F32, tag="st", name=f"st{i}")
                           for i in range(2)]
                    ests = [estp.tile([128, 2 * QH], BF16, tag="est", name=f"est{i}")
                            for i in range(2)]
                    # h-major: each half's row-packed ST pair is adjacent
                    # (concurrent on PE) and its exp fires right after it.
                    # h0 exp on ScalarE; h1 exp on DVE (fast-exp) for the
                    # long/medium slots so both engines run concurrently.
                    wid = 2 * QH if 2 * p + 1 < nreal else QH
                    for h in range(2):
                        nc.tensor.matmul(
                            sts[h][:, 0:QH],
                            kt2[0:64, c0 : c0 + 128],
                            qt2[0:64, h * QH : (h + 1) * QH],
                            start=True,
                            stop=True,
                            tile_position=(0, 0),
                        )
                        if wid == 2 * QH:
                            nc.tensor.matmul(
                                sts[h][:, QH : 2 * QH],
                                kt2[64:128, c1 : c1 + 128],
                                qt2[64:128, h * QH : (h + 1) * QH],
                                start=True,
                                stop=True,
                                tile_position=(64, 0),
                            )
                        if h == 1 and nreal >= 3:
                            nc.vector.tensor_scalar(
                                out=ests[h][:, 0:wid].bitcast(I16),
                                in0=sts[h][:, 0:wid],
                                scalar1=SCHRA_A,
                                scalar2=SCHRA_B,
                                op0=ALU.mult,
                                op1=ALU.add,
                            )
                        else:
                            nc.scalar.activation(
                                out=ests[h][:, 0:wid], in_=sts[h][:, 0:wid],
                                func=AF.Exp, scale=0.125
                            )
                    pendq.append(
                        (emit_ut, (p, ests), None)
                        if p < npairs - 1
                        else (emit_ut, (p, ests), postprocess))
                    if len(pendq) > LOOKAHEAD:
                        pe = pendq.pop(0)
                        pe[0](*pe[1])
                        if pe[2] is not None:
                            pe[2]()
            for pe in pendq:
                pe[0](*pe[1])
                if pe[2] is not None:
                    pe[2]()

    nc.compile()
    return nc


def _plan(valid_lens):
    """Sort batches by length, deal to (slot, core); per-slot chunk counts."""
    order = np.argsort(-valid_lens, kind="stable")  # [B]
    nreals = []
    for s in range(BPC):
        slot_max = int(valid_lens[order[s * NCORES]])
        nreals.append(max(1, -(-slot_max // 128)))  # ceil, >= 1
    return order, tuple(nreals)


def _make_in_maps(query, key, value, valid_lens, order):
    qt = query.transpose(0, 2, 1)  # views [B, D, S]
    kt = key.transpose(0, 2, 1)
    arange_s = np.arange(S)
    in_maps = []
    for c in range(NCORES):
        idx = [int(order[s * NCORES + c]) for s in range(BPC)]
        qt_h = np.empty((BPC, 128, S), dtype=NPBF16)
        kt_h = np.empty((BPC, 128, S), dtype=NPBF16)
        qt_h[:, 0:64] = qt[idx]
        qt_h[:, 64:128] = qt_h[:, 0:64]
        kt_h[:, 0:64] = kt[idx]
        kt_h[:, 64:128] = kt_h[:, 0:64]
        vm_h = np.zeros((BPC, S, D + 1), dtype=NPBF16)
        for s in range(BPC):
            L = int(valid_lens[idx[s]])
            vm_h[s, 0:L, 0:D] = value[idx[s], 0:L]
            vm_h[s, 0:L, D] = 1.0
        in_maps.append({"qt": qt_h, "kt": kt_h, "vm": vm_h})
    return in_maps


def _gather(results, order):
    out = np.empty((B, S, D), dtype=np.float32)
    for c in range(NCORES):
        otc = results[c]["ot"]  # [BPC, D+1, S] unnormalized + denom row
        for s in range(BPC):
            o = otc[s]
            out[int(order[s * NCORES + c])] = (o[0:D] / o[D : D + 1]).T
    return out


def kernel(query, key, value, valid_lens):
    query = np.ascontiguousarray(np.asarray(query, dtype=np.float32))
    key = np.ascontiguousarray(np.asarray(key, dtype=np.float32))
    value = np.ascontiguousarray(np.asarray(value, dtype=np.float32))
    valid_lens = np.asarray(valid_lens).astype(np.int32).reshape(B)
    assert query.shape == (B, S, D) and key.shape == (B, S, D)
    assert value.shape == (B, S, D)

    order, nreals = _plan(valid_lens)
    nc = _BUILD_CACHE.get(nreals)
    if nc is None:
        nc = _build(nreals)
        _BUILD_CACHE[nreals] = nc

    in_maps = _make_in_maps(query, key, value, valid_lens, order)
    res = bass_utils.run_bass_kernel_spmd(nc, in_maps, core_ids=list(range(NCORES)))
    return _gather(res.results, order)


# revision 33
# speedup vs baseline: 1.0306x; 1.0306x over previous
"""Trainium2 Bass kernel for nn_DotProductAttention_6030134084023.

reference: softmax(mask(Q @ K^T / sqrt(64), valid_lens)) @ V
  query/key/value: [64, 1024, 64] f32, valid_lens: [64] int32 -> [64, 1024, 64] f32

Strategy
--------
Batch dim sharded across the 8 NeuronCores. The host sorts batches by
valid_len (descending) and deals them round-robin, so slot s on every core
holds similar-length batches; the kernel is compiled per call with a
per-slot chunk count (chunks past a slot's max length have an all-zero
mask so skipping them is exactly lossless; the kernel recompiles for
whatever valid_lens it receives — correctness never depends on the
specialization).

All matmuls run in bf16 (1 cycle/row on the PE vs 3 for fp32 HIGH mode);
inputs are cast and laid out on the host:

  qt/kt: [BPC, 128, S] bf16, Q^T/K^T duplicated into both 64-row halves
         so row-packed K=64 matmul pairs (tile_position (0,0)/(64,0))
         run concurrently from one SBUF tile, one DMA per tensor.
  vm:    [BPC, S, 65] bf16 = [V * mask, mask] — the valid_lens mask is
         applied to V on the host, so scores need no masking on device
         and UT row 64 accumulates the masked softmax denominator.

Per-core dataflow per batch in the "S^T orientation" (k on SBUF
partitions, q on the free dim; no on-device transposes):

  ST[k, q]  = KT_chunk.T @ QT      PE bf16; two k-chunks concurrently
  EST       = exp(0.125 * ST)      one ScalarE op per [128, 1024] group
  UT[d', q] += Vm_chunk.T @ EST    PE bf16 K=128, PSUM-accumulated

Postprocess per (batch, q-half): recip = 1/UT[64, :] (DVE, PSUM in),
broadcast recip to 64 partitions via a stride-0-partition SBUF DMA,
OT = UT[0:64, :] * bc (DVE), one output DMA per batch.
"""

import numpy as np
import ml_dtypes

import concourse.bass as bass
import concourse.bacc as bacc
import concourse.tile as tile
from concourse import mybir
from concourse import bass_utils

F32 = mybir.dt.float32
BF16 = mybir.dt.bfloat16
I16 = mybir.dt.int16
AF = mybir.ActivationFunctionType
ALU = mybir.AluOpType

# Schraudolph fast-exp constants emitting bf16 bit patterns via the DVE's
# round-to-nearest f32->i16 convert: exp(x/8) ~= bf16_bits(round(x*A + B)).
# Max ~3% rel error; applied only to long-slot score groups where the
# softmax is diffuse and the error washes out (measured 5.6e-3 end to end).
SCHRA_A = float(128.0 * np.log2(np.e) / 8.0)
SCHRA_B = float(16256.0 - 4.5)

NCORES = 8
B = 64
S = 1024
D = 64
BPC = B // NCORES  # 8 batch slots per core
KC = S // 128  # 8 k-chunks of 128
QH = 512  # q-half (max matmul moving dim per PSUM bank)

NPBF16 = ml_dtypes.bfloat16

_BUILD_CACHE = {}


def _build(nreals):
    """nreals[s]: number of 128-key chunks with any valid key for slot s."""
    nc = bacc.Bacc("TRN2", target_bir_lowering=False, debug=False, num_devices=NCORES)
    qt = nc.dram_tensor("qt", [BPC, 128, S], BF16, kind="ExternalInput").ap()
    kt = nc.dram_tensor("kt", [BPC, 128, S], BF16, kind="ExternalInput").ap()
    vm = nc.dram_tensor("vm", [BPC, S, D + 1], BF16, kind="ExternalInput").ap()
    # unnormalized output: rows 0:64 = V'^T @ EST, row 64 = softmax denom;
    # the host divides (free) during the gather
    ot = nc.dram_tensor("ot", [BPC, D + 1, S], F32, kind="ExternalOutput").ap()

    with tile.TileContext(nc) as tc:
        with (
            tc.tile_pool(name="qk", bufs=3) as qkp,
            tc.tile_pool(name="vmp", bufs=3) as vmp,
            tc.tile_pool(name="estp", bufs=8) as estp,
            tc.tile_pool(name="post", bufs=3) as postp,
            tc.tile_pool(name="stp", bufs=3, space="PSUM") as stp,
            tc.tile_pool(name="utp", bufs=1, space="PSUM") as utp,
        ):
            # alternate long/short slots so short-batch postprocess tails
            # hide under the next long batch's compute; end with shortest
            slot_order = [5, 0, 4, 1, 6, 2, 3, 7]
            # STs run LOOKAHEAD pairs ahead of the est-waiting UTs in PE's
            # in-order queue, so both exp engines (ACT + DVE) stay fed
            LOOKAHEAD = 1
            pendq = []
            for bi, b in enumerate(slot_order):
                nreal = nreals[b]
                kw = nreal * 128
                qt2 = qkp.tile([128, S], BF16, tag="qt")
                kt2 = qkp.tile([128, S], BF16, tag="kt")
                # k first, then q; split the first slot's q so its first ST
                # pair starts as early as possible
                nc.sync.dma_start(out=kt2[:, 0:kw], in_=kt[b, :, 0:kw])
                if bi == 0:
                    nc.sync.dma_start(out=qt2[:, 0:QH], in_=qt[b, :, 0:QH])
                    nc.sync.dma_start(out=qt2[:, QH:S], in_=qt[b, :, QH:S])
                else:
                    nc.sync.dma_start(out=qt2[:], in_=qt[b])

                # all V' chunks of the batch in one strided DMA:
                # vm_all[p, kc*65 + j] = vm[b, kc*128 + p, j]
                vm_all = vmp.tile([128, nreal * (D + 1)], BF16, tag="vm")
                vsrc = vm[b]  # [S, D+1]
                nc.sync.dma_start(
                    out=vm_all[:],
                    in_=bass.AP(
                        tensor=vsrc.tensor,
                        offset=vsrc.offset,
                        ap=[[D + 1, 128], [128 * (D + 1), nreal], [1, D + 1]],
                    ),
                )

                npairs = (nreal + 1) // 2
                uts = [utp.tile([D + 1, QH], F32, tag=f"ut{h}", name=f"ut{h}")
                       for h in range(2)]

                def emit_ut(p, ests, uts=uts, nreal=nreal, vm_all=vm_all):
                    # both halves back-to-back per chunk: shared vm weights
                    for kcl in range(2):
                        kc = 2 * p + kcl
                        if kc >= nreal:
                            continue
                        for h in range(2):
                            nc.tensor.matmul(
                                uts[h][:],
                                vm_all[:, kc * (D + 1) : (kc + 1) * (D + 1)],
                                ests[h][:, kcl * QH : (kcl + 1) * QH],
                                start=(p == 0 and kcl == 0),
                                stop=(kc == nreal - 1),
                            )

                def postprocess(uts=uts, b=b):
                    # one PSUM->SBUF eviction per engine so neither exp
                    # queue eats both copies at the batch boundary
                    for h in range(2):
                        hs = slice(h * QH, (h + 1) * QH)
                        osb = postp.tile([D + 1, QH], F32, tag=f"osb{h}")
                        if h == 0:
                            nc.scalar.copy(out=osb[:], in_=uts[h][:])
                        else:
                            nc.vector.tensor_copy(out=osb[:], in_=uts[h][:])
                        nc.sync.dma_start(out=ot[b, :, hs], in_=osb[:])

                for p in range(npairs):
                    c0 = 2 * p * 128
                    c1 = (2 * p + 1) * 128
                    sts = [stp.tile([128, 2 * QH], F32, tag="st", name=f"st{i}")
                           for i in range(2)]
                    ests = [estp.tile([128, 2 * QH], BF16, tag="est", name=f"est{i}")
                            for i in range(2)]
                    # h-major: each half's row-packed ST pair is adjacent
                    # (concurrent on PE) and its exp fires right after it.
                    # h0 exp on ScalarE; h1 exp on DVE (fast-exp) for the
                    # long/medium slots so both engines run concurrently.
                    wid = 2 * QH if 2 * p + 1 < nreal else QH
                    for h in range(2):
                        nc.tensor.matmul(
                            sts[h][:, 0:QH],
                            kt2[0:64, c0 : c0 + 128],
                            qt2[0:64, h * QH : (h + 1) * QH],
                            start=True,
                            stop=True,
                            tile_position=(0, 0),
                        )
                        if wid == 2 * QH:
                            nc.tensor.matmul(
                                sts[h][:, QH : 2 * QH],
                                kt2[64:128, c1 : c1 + 128],
                                qt2[64:128, h * QH : (h + 1) * QH],
                                start=True,
                                stop=True,
                                tile_position=(64, 0),
                            )
                        if h == 1 and nreal >= 3:
                            nc.vector.tensor_scalar(
                                out=ests[h][:, 0:wid].bitcast(I16),
                                in0=sts[h][:, 0:wid],
                                scalar1=SCHRA_A,
                                scalar2=SCHRA_B,
                                op0=ALU.mult,
                                op1=ALU.add,
                            )
                        else:
                            nc.scalar.activation(
                                out=ests[h][:, 0:wid], in_=sts[h][:, 0:wid],
                                func=AF.Exp, scale=0.125
                            )
                    pendq.append(
                        (emit_ut, (p, ests), None)
                        if p < npairs - 1
                        else (emit_ut, (p, ests), postprocess))
                    if len(pendq) > LOOKAHEAD:
                        pe = pendq.pop(0)
                        pe[0](*pe[1])
                        if pe[2] is not None:
                            pe[2]()
            for pe in pendq:
                pe[0](*pe[1])
                if pe[2] is not None:
                    pe[2]()

    nc.compile()
    return nc


def _plan(valid_lens):
    """Sort batches by length, deal to (slot, core); per-slot chunk counts."""
    order = np.argsort(-valid_lens, kind="stable")  # [B]
    nreals = []
    for s in range(BPC):
        slot_max = int(valid_lens[order[s * NCORES]])
        nreals.append(max(1, -(-slot_max // 128)))  # ceil, >= 1
    return order, tuple(nreals)


def _make_in_maps(query, key, value, valid_lens, order):
    qt = query.transpose(0, 2, 1)  # views [B, D, S]
    kt = key.transpose(0, 2, 1)
    arange_s = np.arange(S)
    in_maps = []
    for c in range(NCORES):
        idx = [int(order[s * NCORES + c]) for s in range(BPC)]
        qt_h = np.empty((BPC, 128, S), dtype=NPBF16)
        kt_h = np.empty((BPC, 128, S), dtype=NPBF16)
        qt_h[:, 0:64] = qt[idx]
        qt_h[:, 64:128] = qt_h[:, 0:64]
        kt_h[:, 0:64] = kt[idx]
        kt_h[:, 64:128] = kt_h[:, 0:64]
        vm_h = np.zeros((BPC, S, D + 1), dtype=NPBF16)
        for s in range(BPC):
            L = int(valid_lens[idx[s]])
            vm_h[s, 0:L, 0:D] = value[idx[s], 0:L]
            vm_h[s, 0:L, D] = 1.0
        in_maps.append({"qt": qt_h, "kt": kt_h, "vm": vm_h})
    return in_maps


def _gather(results, order):
    out = np.empty((B, S, D), dtype=np.float32)
    for c in range(NCORES):
        otc = results[c]["ot"]  # [BPC, D+1, S] unnormalized + denom row
        for s in range(BPC):
            o = otc[s]
            out[int(order[s * NCORES + c])] = (o[0:D] / o[D : D + 1]).T
    return out


def kernel(query, key, value, valid_lens):
    query = np.ascontiguousarray(np.asarray(query, dtype=np.float32))
    key = np.ascontiguousarray(np.asarray(key, dtype=np.float32))
    value = np.ascontiguousarray(np.asarray(value, dtype=np.float32))
    valid_lens = np.asarray(valid_lens).astype(np.int32).reshape(B)
    assert query.shape == (B, S, D) and key.shape == (B, S, D)
    assert value.shape == (B, S, D)

    order, nreals = _plan(valid_lens)
    nc = _BUILD_CACHE.get(nreals)
    if nc is None:
        nc = _build(nreals)
        _BUILD_CACHE[nreals] = nc

    in_maps = _make_in_maps(query, key, value, valid_lens, order)
    res = bass_utils.run_bass_kernel_spmd(nc, in_maps, core_ids=list(range(NCORES)))
    return _gather(res.results, order)
